# revision 12
# baseline (speedup 1.0000x reference)
"""GQA attention (B=2, S=2048, DIM=2048, H=16, KVH=4, HD=128, RoPE, causal)
on 8 TRN2 NeuronCores.

Sharding: core c -> batch b = c//4, head-group g = c%4 (q heads 4g..4g+3,
which map exactly to kv head g). Each core computes the partial output
attn_heads @ wo_slice.T  ([S, DIM]); the host sums the 4 partials per batch.

Device layout (everything "transposed", feature-major):
  xT   [DIM, S]   bf16   x[b].T
  wqT  [DIM, 512] bf16   (per-head even/odd-permuted, 1/sqrt(HD)-scaled) wq.T
  wkT  [DIM, 128] bf16   permuted wk.T
  wvT  [DIM, 128] bf16   wv.T (not permuted; v is not roped)
  woT  [512, DIM] bf16   wo[:, cols].T
  cosT/sinT [64, S] f32  rope tables, frequency-major

The per-head even/odd permutation (rows [0,2,..,126,1,3,..,127]) turns RoPE
pair-interleaving into contiguous half-partitions; q.k dot products are
invariant because q and k are permuted identically.

Attention is computed in transposed score layout: scoresT[k, q] so that
probsT feeds the PV matmul directly (lhsT = v natural layout), attnT falls
out in [hd, q] = exactly the lhsT the output projection needs, and the
softmax denominators come from an all-ones stationary matmul.
"""

import math
import sys
import types

import numpy as np

try:
    import concourse.bacc as bacc  # noqa: F401
except ImportError:
    sys.path.insert(0, "/opt/trn_rl_repo")

import ml_dtypes
import concourse.bacc as bacc
import concourse.tile as tile
from concourse import mybir
from concourse.bass_utils import run_bass_kernel_spmd
from concourse.bass import _add_dep_helper

BF16 = mybir.dt.bfloat16
F32 = mybir.dt.float32

B, S, DIM = 2, 2048, 2048
H, KVH, HD = 16, 4, 128
N_CORES = 8
P = 128
D_T = DIM // P      # 16 contraction tiles
NH = H // KVH       # 4 q-heads per core
QC = 512            # q-chunk (matmul moving free dim)
QB = S // QC        # 4 q-chunks
S_T = S // P        # 16 s-tiles / k-tiles
NEG = -1.0e9

_cached = {}


def _build_nc():
    nc = bacc.Bacc("TRN2", target_bir_lowering=False, debug=False,
                   num_devices=N_CORES)
    xT = nc.dram_tensor("xT", [DIM, S], BF16, kind="ExternalInput").ap()
    wqT = nc.dram_tensor("wqT", [DIM, NH * HD], BF16, kind="ExternalInput").ap()
    wkT = nc.dram_tensor("wkT", [DIM, HD], BF16, kind="ExternalInput").ap()
    wvT = nc.dram_tensor("wvT", [DIM, HD], BF16, kind="ExternalInput").ap()
    woT = nc.dram_tensor("woT", [NH * HD, DIM], BF16, kind="ExternalInput").ap()
    cosT = nc.dram_tensor("cosT", [HD, S], BF16, kind="ExternalInput").ap()
    sinT = nc.dram_tensor("sinT", [HD, S], BF16, kind="ExternalInput").ap()
    out = nc.dram_tensor("out", [S, DIM], BF16, kind="ExternalOutput").ap()

    with tile.TileContext(nc) as tc:
        _build_kernel(tc, xT, wqT, wkT, wvT, woT, cosT, sinT, out)
    nc.compile()
    return nc


def _build_kernel(tc, xT, wqT, wkT, wvT, woT, cosT, sinT, out):
    nc = tc.nc
    Exp = mybir.ActivationFunctionType.Exp

    with (
        tc.tile_pool(name="const", bufs=1) as const,
        tc.tile_pool(name="big", bufs=1) as big,
        tc.tile_pool(name="rtmp", bufs=8) as rtmp,
        tc.tile_pool(name="probs", bufs=8) as probs_pool,
        tc.tile_pool(name="attn", bufs=8) as attn_pool,
        tc.tile_pool(name="rz", bufs=2) as rz_pool,
        tc.tile_pool(name="osb", bufs=4) as osb_pool,
        tc.tile_pool(name="ps", bufs=4, space="PSUM") as ps_pool,
        tc.tile_pool(name="ps_at", bufs=2, space="PSUM") as ps_at_pool,
        tc.tile_pool(name="ps_z", bufs=2, space="PSUM") as ps_z_pool,
    ):
        # ---- constants ----
        ones = const.tile([P, P], BF16, name="ones")
        nc.vector.memset(ones, 1.0)
        # cos_sb = [cos; cos], sin_sb = [-sin; sin] (host-prepared), so the
        # whole rotation is 3 full-width ops on partition-aligned tiles.
        cos_sb = const.tile([HD, S], BF16, name="cos")
        sin_sb = const.tile([HD, S], BF16, name="sin")
        nc.sync.dma_start(out=cos_sb, in_=cosT)
        nc.sync.dma_start(out=sin_sb, in_=sinT)

        # ---- weights / activations ----
        wq_sb = big.tile([P, D_T, NH * HD], BF16, name="wq")
        nc.sync.dma_start(out=wq_sb, in_=wqT.rearrange("(t p) j -> p t j", p=P))
        wk_sb = big.tile([P, D_T, HD], BF16, name="wk")
        nc.sync.dma_start(out=wk_sb, in_=wkT.rearrange("(t p) j -> p t j", p=P))
        wv_sb = big.tile([P, D_T, HD], BF16, name="wv")
        nc.sync.dma_start(out=wv_sb, in_=wvT.rearrange("(t p) j -> p t j", p=P))
        wo_sb = big.tile([P, NH, DIM], BF16, name="wo")
        nc.sync.dma_start(out=wo_sb, in_=woT.rearrange("(t p) d -> p t d", p=P))

        # One tile per d-slice so projection matmuls only wait on their own
        # slice's DMA, not the whole 8 MB load.
        xt_tiles = {}
        xt_dmas = []
        for dt in range(D_T):
            t = big.tile([P, S], BF16, name=f"xt{dt}")
            dma = nc.sync.dma_start(out=t, in_=xT[dt * P:(dt + 1) * P, :])
            if dt >= 8:
                _add_dep_helper(dma.ins, xt_dmas[dt - 8].ins, sync=True,
                                reason="stagger xT load")
            for sc in range(QB):
                xt_tiles[(dt, sc)] = t[:, sc * QC:(sc + 1) * QC]
            xt_dmas.append(dma)

        qT = big.tile([P, NH, S], BF16, name="qT")
        kT = big.tile([P, S], BF16, name="kT")
        v_sb = big.tile([P, S_T, HD], BF16, name="v")

        def rope(dst, ps, sc):
            """dst (bf16 [128,512] slice) <- rotate(ps).

            ACT stages ps to bf16 SBUF twice (straight + halves swapped via
            ScalarE partition-shifting copies); DVE then runs three
            full-width 16-bit 2x-mode ops against the sign-folded tables:
            dst = st*[cos;cos] + sw*[-sin;sin]."""
            h = HD // 2
            st = rtmp.tile([P, QC], BF16, name="rst")
            sw = rtmp.tile([P, QC], BF16, name="rsw")
            nc.scalar.copy(out=st, in_=ps)
            nc.gpsimd.tensor_copy(out=sw[0:h, :], in_=st[h:P, :])
            nc.gpsimd.tensor_copy(out=sw[h:P, :], in_=st[0:h, :])
            cos_c = cos_sb[:, sc * QC:(sc + 1) * QC]
            sin_c = sin_sb[:, sc * QC:(sc + 1) * QC]
            t0 = rtmp.tile([P, QC], BF16, name="rt")
            t1 = rtmp.tile([P, QC], BF16, name="rt")
            nc.vector.tensor_mul(t0, st, cos_c)
            nc.vector.tensor_mul(t1, sw, sin_c)
            nc.vector.tensor_add(dst, t0, t1)

        # K projection + rope
        for sc in range(QB):
            ps = ps_pool.tile([P, QC], F32, name="ps")
            for dt in range(D_T):
                nc.tensor.matmul(ps, lhsT=wk_sb[:, dt, :],
                                 rhs=xt_tiles[(dt, sc)],
                                 start=(dt == 0), stop=(dt == D_T - 1))
            rope(kT[:, sc * QC:(sc + 1) * QC], ps, sc)

        # Q projection + rope
        for hh in range(NH):
            for sc in range(QB):
                ps = ps_pool.tile([P, QC], F32, name="ps")
                for dt in range(D_T):
                    nc.tensor.matmul(ps, lhsT=wq_sb[:, dt, hh * HD:(hh + 1) * HD],
                                     rhs=xt_tiles[(dt, sc)],
                                     start=(dt == 0), stop=(dt == D_T - 1))
                rope(qT[:, hh, sc * QC:(sc + 1) * QC], ps, sc)

        # V projection (natural [s, hd] layout)
        for st in range(S_T):
            ps = ps_pool.tile([P, QC], F32, name="ps")
            for dt in range(D_T):
                nc.tensor.matmul(ps[:, 0:HD],
                                 lhsT=xt_tiles[(dt, st // 4)][:, (st % 4) * P:(st % 4 + 1) * P],
                                 rhs=wv_sb[:, dt, :],
                                 start=(dt == 0), stop=(dt == D_T - 1))
            nc.scalar.copy(out=v_sb[:, st, :], in_=ps[:, 0:HD])

        # ---- attention + output projection, per q-chunk ----
        for qb in reversed(range(QB)):
            nk = (qb + 1) * (QC // P)  # causal k-tiles for this q-chunk
            attn_tiles = []
            for hh in range(NH):
                at_ps = ps_at_pool.tile([P, QC], F32, name="at")
                z_ps = ps_z_pool.tile([P, QC], F32, name="z")
                for k in range(nk):
                    di = k - qb * (QC // P)
                    # On diagonal tiles only columns q0+128*di.. are causally
                    # valid; narrow every stage to that width. off=0 for all
                    # sub-diagonal tiles.
                    off = P * di if di > 0 else 0
                    w = QC - off
                    sc_ps = ps_pool.tile([P, QC], F32, name="ps")
                    nc.tensor.matmul(sc_ps[:, 0:w], lhsT=kT[:, k * P:(k + 1) * P],
                                     rhs=qT[:, hh, qb * QC + off:(qb + 1) * QC],
                                     start=True, stop=True)
                    pr = probs_pool.tile([P, QC], BF16, name="pr")
                    nc.scalar.activation(out=pr[:, 0:w], in_=sc_ps[:, 0:w],
                                         func=Exp)
                    if di >= 0:  # diagonal tile -> zero where c' < r
                        nc.gpsimd.affine_select(
                            out=pr[:, 0:w], in_=pr[:, 0:w],
                            compare_op=mybir.AluOpType.is_ge,
                            fill=0.0, base=0, pattern=[[1, w]],
                            channel_multiplier=-1)
                    nc.tensor.matmul(at_ps[:, off:QC], lhsT=v_sb[:, k, :],
                                     rhs=pr[:, 0:w],
                                     start=(k == 0), stop=(k == nk - 1))
                    nc.tensor.matmul(z_ps[:, off:QC], lhsT=ones, rhs=pr[:, 0:w],
                                     start=(k == 0), stop=(k == nk - 1))
                rz = rz_pool.tile([P, QC], F32, name="rz")
                nc.vector.reciprocal_approx_fast(out=rz, in_=z_ps)
                a_sb = attn_pool.tile([P, QC], BF16, name="attn")
                nc.vector.tensor_mul(a_sb, at_ps, rz)
                attn_tiles.append(a_sb)

            for st in range(QC // P):
                row0 = qb * QC + st * P
                for dc in range(DIM // QC):
                    op_ps = ps_pool.tile([P, QC], F32, name="ps")
                    for j in range(NH):
                        nc.tensor.matmul(
                            op_ps, lhsT=attn_tiles[j][:, st * P:(st + 1) * P],
                            rhs=wo_sb[:, j, dc * QC:(dc + 1) * QC],
                            start=(j == 0), stop=(j == NH - 1))
                    o_sb = osb_pool.tile([P, QC], BF16, name="osb")
                    if (st + dc) % 2 == 0:
                        nc.scalar.copy(out=o_sb, in_=op_ps)
                    else:
                        nc.vector.tensor_copy(out=o_sb, in_=op_ps)
                    nc.sync.dma_start(
                        out=out[row0:row0 + P, dc * QC:(dc + 1) * QC], in_=o_sb)


def _get_nc():
    if "nc" not in _cached:
        _cached["nc"] = _build_nc()
    return _cached["nc"]


def _prep_in_maps(x, freqs_cis, wq, wk, wv, wo):
    bf = ml_dtypes.bfloat16
    perm = np.concatenate([np.arange(0, HD, 2), np.arange(1, HD, 2)])
    scale = 1.0 / math.sqrt(HD)
    wq_p = (wq.reshape(H, HD, DIM)[:, perm, :] * scale).astype(np.float32)
    wk_p = wk.reshape(KVH, HD, DIM)[:, perm, :]
    cos_h = np.ascontiguousarray(freqs_cis[:, :, 0].T)  # [64, S]
    sin_h = np.ascontiguousarray(freqs_cis[:, :, 1].T)
    cosT = np.concatenate([cos_h, cos_h], axis=0).astype(bf)   # [128, S]
    sinT = np.concatenate([-sin_h, sin_h], axis=0).astype(bf)

    in_maps = []
    for c in range(N_CORES):
        b, g = c // KVH, c % KVH
        hq = slice(NH * g, NH * (g + 1))
        in_maps.append({
            "xT": np.ascontiguousarray(x[b].T).astype(bf),
            "wqT": np.ascontiguousarray(
                wq_p[hq].reshape(NH * HD, DIM).T).astype(bf),
            "wkT": np.ascontiguousarray(wk_p[g].T).astype(bf),
            "wvT": np.ascontiguousarray(wv[g * HD:(g + 1) * HD].T).astype(bf),
            "woT": np.ascontiguousarray(
                wo[:, NH * HD * g:NH * HD * (g + 1)].T).astype(bf),
            "cosT": cosT,
            "sinT": sinT,
        })
    return in_maps


def _reduce_outputs(results):
    out = np.zeros((B, S, DIM), np.float32)
    for c in range(N_CORES):
        out[c // KVH] += results[c]["out"].astype(np.float32)
    return out


def kernel(x, freqs_cis, wq, wk, wv, wo, _trace=False, _trace_kwargs=None):
    nc = _get_nc()
    in_maps = _prep_in_maps(x, freqs_cis, wq, wk, wv, wo)
    res = run_bass_kernel_spmd(nc, in_maps, core_ids=list(range(N_CORES)),
                               trace=_trace, **(_trace_kwargs or {}))
    out = _reduce_outputs(res.results)
    if _trace:
        _cached["last_exec_time_ns"] = res.exec_time_ns
        _cached["last_results"] = res
    return out


# revision 13
# speedup vs baseline: 1.0122x; 1.0122x over previous
"""GQA attention (B=2, S=2048, DIM=2048, H=16, KVH=4, HD=128, RoPE, causal)
on 8 TRN2 NeuronCores.

Sharding: core c -> batch b = c//4, head-group g = c%4 (q heads 4g..4g+3,
which map exactly to kv head g). Each core computes the partial output
attn_heads @ wo_slice.T  ([S, DIM]); the host sums the 4 partials per batch.

Device layout (everything "transposed", feature-major):
  xT   [DIM, S]   bf16   x[b].T
  wqT  [DIM, 512] bf16   (per-head even/odd-permuted, 1/sqrt(HD)-scaled) wq.T
  wkT  [DIM, 128] bf16   permuted wk.T
  wvT  [DIM, 128] bf16   wv.T (not permuted; v is not roped)
  woT  [512, DIM] bf16   wo[:, cols].T
  cosT/sinT [64, S] f32  rope tables, frequency-major

The per-head even/odd permutation (rows [0,2,..,126,1,3,..,127]) turns RoPE
pair-interleaving into contiguous half-partitions; q.k dot products are
invariant because q and k are permuted identically.

Attention is computed in transposed score layout: scoresT[k, q] so that
probsT feeds the PV matmul directly (lhsT = v natural layout), attnT falls
out in [hd, q] = exactly the lhsT the output projection needs, and the
softmax denominators come from an all-ones stationary matmul.
"""

import math
import sys
import types

import numpy as np

try:
    import concourse.bacc as bacc  # noqa: F401
except ImportError:
    sys.path.insert(0, "/opt/trn_rl_repo")

import ml_dtypes
import concourse.bacc as bacc
import concourse.tile as tile
from concourse import mybir
from concourse.bass_utils import run_bass_kernel_spmd
from concourse.bass import _add_dep_helper

BF16 = mybir.dt.bfloat16
F32 = mybir.dt.float32

B, S, DIM = 2, 2048, 2048
H, KVH, HD = 16, 4, 128
N_CORES = 8
P = 128
D_T = DIM // P      # 16 contraction tiles
NH = H // KVH       # 4 q-heads per core
QC = 512            # q-chunk (matmul moving free dim)
QB = S // QC        # 4 q-chunks
S_T = S // P        # 16 s-tiles / k-tiles
NEG = -1.0e9

_cached = {}


def _build_nc():
    nc = bacc.Bacc("TRN2", target_bir_lowering=False, debug=False,
                   num_devices=N_CORES)
    xT = nc.dram_tensor("xT", [DIM, S], BF16, kind="ExternalInput").ap()
    wqT = nc.dram_tensor("wqT", [DIM, NH * HD], BF16, kind="ExternalInput").ap()
    wkT = nc.dram_tensor("wkT", [DIM, HD], BF16, kind="ExternalInput").ap()
    wvT = nc.dram_tensor("wvT", [DIM, HD], BF16, kind="ExternalInput").ap()
    woT = nc.dram_tensor("woT", [NH * HD, DIM], BF16, kind="ExternalInput").ap()
    cosT = nc.dram_tensor("cosT", [HD, S], BF16, kind="ExternalInput").ap()
    sinT = nc.dram_tensor("sinT", [HD, S], BF16, kind="ExternalInput").ap()
    out = nc.dram_tensor("out", [S, DIM], BF16, kind="ExternalOutput").ap()

    with tile.TileContext(nc) as tc:
        _build_kernel(tc, xT, wqT, wkT, wvT, woT, cosT, sinT, out)
    nc.compile()
    return nc


def _build_kernel(tc, xT, wqT, wkT, wvT, woT, cosT, sinT, out):
    nc = tc.nc
    Exp = mybir.ActivationFunctionType.Exp

    with (
        tc.tile_pool(name="const", bufs=1) as const,
        tc.tile_pool(name="big", bufs=1) as big,
        tc.tile_pool(name="rtmp", bufs=8) as rtmp,
        tc.tile_pool(name="probs", bufs=8) as probs_pool,
        tc.tile_pool(name="attn", bufs=8) as attn_pool,
        tc.tile_pool(name="rz", bufs=2) as rz_pool,
        tc.tile_pool(name="osb", bufs=4) as osb_pool,
        tc.tile_pool(name="ps", bufs=4, space="PSUM") as ps_pool,
        tc.tile_pool(name="ps_at", bufs=2, space="PSUM") as ps_at_pool,
        tc.tile_pool(name="ps_z", bufs=2, space="PSUM") as ps_z_pool,
    ):
        # ---- constants ----
        ones = const.tile([P, P], BF16, name="ones")
        nc.vector.memset(ones, 1.0)
        # cos_sb = [cos; cos], sin_sb = [-sin; sin] (host-prepared), so the
        # whole rotation is 3 full-width ops on partition-aligned tiles.
        cos_sb = const.tile([HD, S], BF16, name="cos")
        sin_sb = const.tile([HD, S], BF16, name="sin")
        nc.sync.dma_start(out=cos_sb, in_=cosT)
        nc.sync.dma_start(out=sin_sb, in_=sinT)

        # ---- weights / activations ----
        wq_sb = big.tile([P, D_T, NH * HD], BF16, name="wq")
        nc.sync.dma_start(out=wq_sb, in_=wqT.rearrange("(t p) j -> p t j", p=P))
        wk_sb = big.tile([P, D_T, HD], BF16, name="wk")
        nc.sync.dma_start(out=wk_sb, in_=wkT.rearrange("(t p) j -> p t j", p=P))
        wv_sb = big.tile([P, D_T, HD], BF16, name="wv")
        nc.sync.dma_start(out=wv_sb, in_=wvT.rearrange("(t p) j -> p t j", p=P))
        wo_sb = big.tile([P, NH, DIM], BF16, name="wo")
        nc.sync.dma_start(out=wo_sb, in_=woT.rearrange("(t p) d -> p t d", p=P))

        # One tile per d-slice so projection matmuls only wait on their own
        # slice's DMA, not the whole 8 MB load.
        xt_tiles = {}
        xt_dmas = []
        for dt in range(D_T):
            t = big.tile([P, S], BF16, name=f"xt{dt}")
            dma = nc.sync.dma_start(out=t, in_=xT[dt * P:(dt + 1) * P, :])
            if dt >= 8:
                _add_dep_helper(dma.ins, xt_dmas[dt - 8].ins, sync=True,
                                reason="stagger xT load")
            for sc in range(QB):
                xt_tiles[(dt, sc)] = t[:, sc * QC:(sc + 1) * QC]
            xt_dmas.append(dma)

        qT = big.tile([P, NH, S], BF16, name="qT")
        kT = big.tile([P, S], BF16, name="kT")
        v_sb = big.tile([P, S_T, HD], BF16, name="v")

        def rope(dst, ps, sc):
            """dst (bf16 [128,512] slice) <- rotate(ps).

            ACT stages ps to bf16 SBUF twice (straight + halves swapped via
            ScalarE partition-shifting copies); DVE then runs three
            full-width 16-bit 2x-mode ops against the sign-folded tables:
            dst = st*[cos;cos] + sw*[-sin;sin]."""
            h = HD // 2
            st = rtmp.tile([P, QC], BF16, name="rst")
            sw = rtmp.tile([P, QC], BF16, name="rsw")
            nc.scalar.copy(out=st, in_=ps)
            nc.gpsimd.tensor_copy(out=sw[0:h, :], in_=st[h:P, :])
            nc.gpsimd.tensor_copy(out=sw[h:P, :], in_=st[0:h, :])
            cos_c = cos_sb[:, sc * QC:(sc + 1) * QC]
            sin_c = sin_sb[:, sc * QC:(sc + 1) * QC]
            t0 = rtmp.tile([P, QC], BF16, name="rt")
            t1 = rtmp.tile([P, QC], BF16, name="rt")
            nc.vector.tensor_mul(t0, st, cos_c)
            nc.vector.tensor_mul(t1, sw, sin_c)
            nc.vector.tensor_add(dst, t0, t1)

        # K projection + rope
        for sc in range(QB):
            ps = ps_pool.tile([P, QC], F32, name="ps")
            for dt in range(D_T):
                nc.tensor.matmul(ps, lhsT=wk_sb[:, dt, :],
                                 rhs=xt_tiles[(dt, sc)],
                                 start=(dt == 0), stop=(dt == D_T - 1))
            rope(kT[:, sc * QC:(sc + 1) * QC], ps, sc)

        # Q projection + rope
        for hh in range(NH):
            for sc in range(QB):
                ps = ps_pool.tile([P, QC], F32, name="ps")
                for dt in range(D_T):
                    nc.tensor.matmul(ps, lhsT=wq_sb[:, dt, hh * HD:(hh + 1) * HD],
                                     rhs=xt_tiles[(dt, sc)],
                                     start=(dt == 0), stop=(dt == D_T - 1))
                rope(qT[:, hh, sc * QC:(sc + 1) * QC], ps, sc)

        # V projection (natural [s, hd] layout)
        for st in range(S_T):
            ps = ps_pool.tile([P, QC], F32, name="ps")
            for dt in range(D_T):
                nc.tensor.matmul(ps[:, 0:HD],
                                 lhsT=xt_tiles[(dt, st // 4)][:, (st % 4) * P:(st % 4 + 1) * P],
                                 rhs=wv_sb[:, dt, :],
                                 start=(dt == 0), stop=(dt == D_T - 1))
            nc.scalar.copy(out=v_sb[:, st, :], in_=ps[:, 0:HD])

        # ---- attention + output projection, per q-chunk ----
        for qb in reversed(range(QB)):
            nk = (qb + 1) * (QC // P)  # causal k-tiles for this q-chunk
            attn_tiles = []
            for hh in range(NH):
                at_ps = ps_at_pool.tile([P, QC], F32, name="at")
                z_ps = ps_z_pool.tile([P, QC], F32, name="z")
                for k in range(nk):
                    di = k - qb * (QC // P)
                    # On diagonal tiles only columns q0+128*di.. are causally
                    # valid; narrow every stage to that width. off=0 for all
                    # sub-diagonal tiles.
                    off = P * di if di > 0 else 0
                    w = QC - off
                    sc_ps = ps_pool.tile([P, QC], F32, name="ps")
                    nc.tensor.matmul(sc_ps[:, 0:w], lhsT=kT[:, k * P:(k + 1) * P],
                                     rhs=qT[:, hh, qb * QC + off:(qb + 1) * QC],
                                     start=True, stop=True)
                    pr = probs_pool.tile([P, QC], BF16, name="pr")
                    nc.scalar.activation(out=pr[:, 0:w], in_=sc_ps[:, 0:w],
                                         func=Exp)
                    if di >= 0:  # diagonal tile -> zero where c' < r
                        nc.gpsimd.affine_select(
                            out=pr[:, 0:w], in_=pr[:, 0:w],
                            compare_op=mybir.AluOpType.is_ge,
                            fill=0.0, base=0, pattern=[[1, w]],
                            channel_multiplier=-1)
                    nc.tensor.matmul(at_ps[:, off:QC], lhsT=v_sb[:, k, :],
                                     rhs=pr[:, 0:w],
                                     start=(k == 0), stop=(k == nk - 1))
                    nc.tensor.matmul(z_ps[:, off:QC], lhsT=ones, rhs=pr[:, 0:w],
                                     start=(k == 0), stop=(k == nk - 1))
                rz = rz_pool.tile([P, QC], F32, name="rz")
                nc.vector.reciprocal_approx_fast(out=rz, in_=z_ps)
                a_sb = attn_pool.tile([P, QC], BF16, name="attn")
                nc.vector.tensor_mul(a_sb, at_ps, rz)
                attn_tiles.append(a_sb)

            for st in range(QC // P):
                row0 = qb * QC + st * P
                for dc in range(DIM // QC):
                    op_ps = ps_pool.tile([P, QC], F32, name="ps")
                    for j in range(NH):
                        nc.tensor.matmul(
                            op_ps, lhsT=attn_tiles[j][:, st * P:(st + 1) * P],
                            rhs=wo_sb[:, j, dc * QC:(dc + 1) * QC],
                            start=(j == 0), stop=(j == NH - 1))
                    o_sb = osb_pool.tile([P, QC], BF16, name="osb")
                    nc.vector.tensor_copy(out=o_sb, in_=op_ps)
                    nc.sync.dma_start(
                        out=out[row0:row0 + P, dc * QC:(dc + 1) * QC], in_=o_sb)


def _get_nc():
    if "nc" not in _cached:
        _cached["nc"] = _build_nc()
    return _cached["nc"]


def _prep_in_maps(x, freqs_cis, wq, wk, wv, wo):
    bf = ml_dtypes.bfloat16
    perm = np.concatenate([np.arange(0, HD, 2), np.arange(1, HD, 2)])
    scale = 1.0 / math.sqrt(HD)
    wq_p = (wq.reshape(H, HD, DIM)[:, perm, :] * scale).astype(np.float32)
    wk_p = wk.reshape(KVH, HD, DIM)[:, perm, :]
    cos_h = np.ascontiguousarray(freqs_cis[:, :, 0].T)  # [64, S]
    sin_h = np.ascontiguousarray(freqs_cis[:, :, 1].T)
    cosT = np.concatenate([cos_h, cos_h], axis=0).astype(bf)   # [128, S]
    sinT = np.concatenate([-sin_h, sin_h], axis=0).astype(bf)

    in_maps = []
    for c in range(N_CORES):
        b, g = c // KVH, c % KVH
        hq = slice(NH * g, NH * (g + 1))
        in_maps.append({
            "xT": np.ascontiguousarray(x[b].T).astype(bf),
            "wqT": np.ascontiguousarray(
                wq_p[hq].reshape(NH * HD, DIM).T).astype(bf),
            "wkT": np.ascontiguousarray(wk_p[g].T).astype(bf),
            "wvT": np.ascontiguousarray(wv[g * HD:(g + 1) * HD].T).astype(bf),
            "woT": np.ascontiguousarray(
                wo[:, NH * HD * g:NH * HD * (g + 1)].T).astype(bf),
            "cosT": cosT,
            "sinT": sinT,
        })
    return in_maps


def _reduce_outputs(results):
    out = np.zeros((B, S, DIM), np.float32)
    for c in range(N_CORES):
        out[c // KVH] += results[c]["out"].astype(np.float32)
    return out


def kernel(x, freqs_cis, wq, wk, wv, wo, _trace=False, _trace_kwargs=None):
    nc = _get_nc()
    in_maps = _prep_in_maps(x, freqs_cis, wq, wk, wv, wo)
    res = run_bass_kernel_spmd(nc, in_maps, core_ids=list(range(N_CORES)),
                               trace=_trace, **(_trace_kwargs or {}))
    out = _reduce_outputs(res.results)
    if _trace:
        _cached["last_exec_time_ns"] = res.exec_time_ns
        _cached["last_results"] = res
    return out


# revision 14
# speedup vs baseline: 1.0418x; 1.0292x over previous
"""GQA attention (B=2, S=2048, DIM=2048, H=16, KVH=4, HD=128, RoPE, causal)
on 8 TRN2 NeuronCores.

Sharding: core c -> batch b = c//4, head-group g = c%4 (q heads 4g..4g+3,
which map exactly to kv head g). Each core computes the partial output
attn_heads @ wo_slice.T  ([S, DIM]); the host sums the 4 partials per batch.

Device layout (everything "transposed", feature-major):
  xT   [DIM, S]   bf16   x[b].T
  wqT  [DIM, 512] bf16   (per-head even/odd-permuted, 1/sqrt(HD)-scaled) wq.T
  wkT  [DIM, 128] bf16   permuted wk.T
  wvT  [DIM, 128] bf16   wv.T (not permuted; v is not roped)
  woT  [512, DIM] bf16   wo[:, cols].T
  cosT/sinT [64, S] f32  rope tables, frequency-major

The per-head even/odd permutation (rows [0,2,..,126,1,3,..,127]) turns RoPE
pair-interleaving into contiguous half-partitions; q.k dot products are
invariant because q and k are permuted identically.

Attention is computed in transposed score layout: scoresT[k, q] so that
probsT feeds the PV matmul directly (lhsT = v natural layout), attnT falls
out in [hd, q] = exactly the lhsT the output projection needs, and the
softmax denominators come from an all-ones stationary matmul.
"""

import math
import sys
import types

import numpy as np

try:
    import concourse.bacc as bacc  # noqa: F401
except ImportError:
    sys.path.insert(0, "/opt/trn_rl_repo")

import ml_dtypes
import concourse.bacc as bacc
import concourse.tile as tile
from concourse import mybir
from concourse.bass_utils import run_bass_kernel_spmd
from concourse.bass import _add_dep_helper

BF16 = mybir.dt.bfloat16
F32 = mybir.dt.float32

B, S, DIM = 2, 2048, 2048
H, KVH, HD = 16, 4, 128
N_CORES = 8
P = 128
D_T = DIM // P      # 16 contraction tiles
NH = H // KVH       # 4 q-heads per core
QC = 512            # q-chunk (matmul moving free dim)
QB = S // QC        # 4 q-chunks
S_T = S // P        # 16 s-tiles / k-tiles
NEG = -1.0e9

_cached = {}


def _build_nc():
    nc = bacc.Bacc("TRN2", target_bir_lowering=False, debug=False,
                   num_devices=N_CORES)
    xT = nc.dram_tensor("xT", [DIM, S], BF16, kind="ExternalInput").ap()
    wqT = nc.dram_tensor("wqT", [DIM, NH * HD], BF16, kind="ExternalInput").ap()
    wkT = nc.dram_tensor("wkT", [DIM, HD], BF16, kind="ExternalInput").ap()
    wvT = nc.dram_tensor("wvT", [DIM, HD], BF16, kind="ExternalInput").ap()
    woT = nc.dram_tensor("woT", [NH * HD, DIM], BF16, kind="ExternalInput").ap()
    cosT = nc.dram_tensor("cosT", [HD, S], BF16, kind="ExternalInput").ap()
    sinT = nc.dram_tensor("sinT", [HD, S], BF16, kind="ExternalInput").ap()
    out = nc.dram_tensor("out", [S, DIM], BF16, kind="ExternalOutput").ap()

    with tile.TileContext(nc) as tc:
        _build_kernel(tc, xT, wqT, wkT, wvT, woT, cosT, sinT, out)
    nc.compile()
    return nc


def _build_kernel(tc, xT, wqT, wkT, wvT, woT, cosT, sinT, out):
    nc = tc.nc
    Exp = mybir.ActivationFunctionType.Exp

    with (
        tc.tile_pool(name="const", bufs=1) as const,
        tc.tile_pool(name="big", bufs=1) as big,
        tc.tile_pool(name="rtmp", bufs=8) as rtmp,
        tc.tile_pool(name="probs", bufs=8) as probs_pool,
        tc.tile_pool(name="attn", bufs=8) as attn_pool,
        tc.tile_pool(name="rz", bufs=2) as rz_pool,
        tc.tile_pool(name="osb", bufs=4) as osb_pool,
        tc.tile_pool(name="ps", bufs=4, space="PSUM") as ps_pool,
        tc.tile_pool(name="ps_at", bufs=2, space="PSUM") as ps_at_pool,
        tc.tile_pool(name="ps_z", bufs=2, space="PSUM") as ps_z_pool,
    ):
        # ---- constants ----
        ones = const.tile([P, P], BF16, name="ones")
        nc.vector.memset(ones, 1.0)
        # cos_sb = [cos; cos], sin_sb = [-sin; sin] (host-prepared), so the
        # whole rotation is 3 full-width ops on partition-aligned tiles.
        cos_sb = const.tile([HD, S], BF16, name="cos")
        sin_sb = const.tile([HD, S], BF16, name="sin")
        nc.sync.dma_start(out=cos_sb, in_=cosT)
        nc.sync.dma_start(out=sin_sb, in_=sinT)

        # ---- weights / activations ----
        wq_sb = big.tile([P, D_T, NH * HD], BF16, name="wq")
        nc.sync.dma_start(out=wq_sb, in_=wqT.rearrange("(t p) j -> p t j", p=P))
        wk_sb = big.tile([P, D_T, HD], BF16, name="wk")
        nc.sync.dma_start(out=wk_sb, in_=wkT.rearrange("(t p) j -> p t j", p=P))
        wv_sb = big.tile([P, D_T, HD], BF16, name="wv")
        nc.sync.dma_start(out=wv_sb, in_=wvT.rearrange("(t p) j -> p t j", p=P))
        wo_sb = big.tile([P, NH, DIM], BF16, name="wo")
        nc.sync.dma_start(out=wo_sb, in_=woT.rearrange("(t p) d -> p t d", p=P))

        # One tile per d-slice so projection matmuls only wait on their own
        # slice's DMA, not the whole 8 MB load.
        xt_tiles = {}
        xt_dmas = []
        for dt in range(D_T):
            t = big.tile([P, S], BF16, name=f"xt{dt}")
            dma = nc.sync.dma_start(out=t, in_=xT[dt * P:(dt + 1) * P, :])
            if dt >= 8:
                _add_dep_helper(dma.ins, xt_dmas[dt - 8].ins, sync=True,
                                reason="stagger xT load")
            for sc in range(QB):
                xt_tiles[(dt, sc)] = t[:, sc * QC:(sc + 1) * QC]
            xt_dmas.append(dma)

        qT = big.tile([P, NH, S], BF16, name="qT")
        kT = big.tile([P, S], BF16, name="kT")
        v_sb = big.tile([P, S_T, HD], BF16, name="v")

        def rope(dst, ps, sc):
            """dst (bf16 [128,512] slice) <- rotate(ps).

            ACT stages ps to bf16 SBUF twice (straight + halves swapped via
            ScalarE partition-shifting copies); DVE then runs three
            full-width 16-bit 2x-mode ops against the sign-folded tables:
            dst = st*[cos;cos] + sw*[-sin;sin]."""
            h = HD // 2
            st = rtmp.tile([P, QC], BF16, name="rst")
            sw = rtmp.tile([P, QC], BF16, name="rsw")
            nc.scalar.copy(out=st, in_=ps)
            nc.scalar.copy(out=sw[0:h, :], in_=ps[h:P, :])
            nc.scalar.copy(out=sw[h:P, :], in_=ps[0:h, :])
            cos_c = cos_sb[:, sc * QC:(sc + 1) * QC]
            sin_c = sin_sb[:, sc * QC:(sc + 1) * QC]
            t0 = rtmp.tile([P, QC], BF16, name="rt")
            t1 = rtmp.tile([P, QC], BF16, name="rt")
            nc.vector.tensor_mul(t0, st, cos_c)
            nc.vector.tensor_mul(t1, sw, sin_c)
            nc.vector.tensor_add(dst, t0, t1)

        # K projection + rope
        for sc in range(QB):
            ps = ps_pool.tile([P, QC], F32, name="ps")
            for dt in range(D_T):
                nc.tensor.matmul(ps, lhsT=wk_sb[:, dt, :],
                                 rhs=xt_tiles[(dt, sc)],
                                 start=(dt == 0), stop=(dt == D_T - 1))
            rope(kT[:, sc * QC:(sc + 1) * QC], ps, sc)

        # Q projection + rope
        for hh in range(NH):
            for sc in range(QB):
                ps = ps_pool.tile([P, QC], F32, name="ps")
                for dt in range(D_T):
                    nc.tensor.matmul(ps, lhsT=wq_sb[:, dt, hh * HD:(hh + 1) * HD],
                                     rhs=xt_tiles[(dt, sc)],
                                     start=(dt == 0), stop=(dt == D_T - 1))
                rope(qT[:, hh, sc * QC:(sc + 1) * QC], ps, sc)

        # V projection (natural [s, hd] layout)
        for st in range(S_T):
            ps = ps_pool.tile([P, QC], F32, name="ps")
            for dt in range(D_T):
                nc.tensor.matmul(ps[:, 0:HD],
                                 lhsT=xt_tiles[(dt, st // 4)][:, (st % 4) * P:(st % 4 + 1) * P],
                                 rhs=wv_sb[:, dt, :],
                                 start=(dt == 0), stop=(dt == D_T - 1))
            nc.scalar.copy(out=v_sb[:, st, :], in_=ps[:, 0:HD])

        # ---- attention + output projection, per q-chunk ----
        for qb in reversed(range(QB)):
            nk = (qb + 1) * (QC // P)  # causal k-tiles for this q-chunk
            attn_tiles = []
            for hh in range(NH):
                at_ps = ps_at_pool.tile([P, QC], F32, name="at")
                z_ps = ps_z_pool.tile([P, QC], F32, name="z")
                for k in range(nk):
                    di = k - qb * (QC // P)
                    # On diagonal tiles only columns q0+128*di.. are causally
                    # valid; narrow every stage to that width. off=0 for all
                    # sub-diagonal tiles.
                    off = P * di if di > 0 else 0
                    w = QC - off
                    sc_ps = ps_pool.tile([P, QC], F32, name="ps")
                    nc.tensor.matmul(sc_ps[:, 0:w], lhsT=kT[:, k * P:(k + 1) * P],
                                     rhs=qT[:, hh, qb * QC + off:(qb + 1) * QC],
                                     start=True, stop=True)
                    pr = probs_pool.tile([P, QC], BF16, name="pr")
                    nc.scalar.activation(out=pr[:, 0:w], in_=sc_ps[:, 0:w],
                                         func=Exp)
                    if di >= 0:  # diagonal tile -> zero where c' < r
                        nc.gpsimd.affine_select(
                            out=pr[:, 0:w], in_=pr[:, 0:w],
                            compare_op=mybir.AluOpType.is_ge,
                            fill=0.0, base=0, pattern=[[1, w]],
                            channel_multiplier=-1)
                    nc.tensor.matmul(at_ps[:, off:QC], lhsT=v_sb[:, k, :],
                                     rhs=pr[:, 0:w],
                                     start=(k == 0), stop=(k == nk - 1))
                    nc.tensor.matmul(z_ps[:, off:QC], lhsT=ones, rhs=pr[:, 0:w],
                                     start=(k == 0), stop=(k == nk - 1))
                rz = rz_pool.tile([P, QC], F32, name="rz")
                nc.vector.reciprocal_approx_fast(out=rz, in_=z_ps)
                a_sb = attn_pool.tile([P, QC], BF16, name="attn")
                nc.vector.tensor_mul(a_sb, at_ps, rz)
                attn_tiles.append(a_sb)

            for st in range(QC // P):
                row0 = qb * QC + st * P
                for dc in range(DIM // QC):
                    op_ps = ps_pool.tile([P, QC], F32, name="ps")
                    for j in range(NH):
                        nc.tensor.matmul(
                            op_ps, lhsT=attn_tiles[j][:, st * P:(st + 1) * P],
                            rhs=wo_sb[:, j, dc * QC:(dc + 1) * QC],
                            start=(j == 0), stop=(j == NH - 1))
                    o_sb = osb_pool.tile([P, QC], BF16, name="osb")
                    nc.vector.tensor_copy(out=o_sb, in_=op_ps)
                    nc.sync.dma_start(
                        out=out[row0:row0 + P, dc * QC:(dc + 1) * QC], in_=o_sb)


def _get_nc():
    if "nc" not in _cached:
        _cached["nc"] = _build_nc()
    return _cached["nc"]


def _prep_in_maps(x, freqs_cis, wq, wk, wv, wo):
    bf = ml_dtypes.bfloat16
    perm = np.concatenate([np.arange(0, HD, 2), np.arange(1, HD, 2)])
    scale = 1.0 / math.sqrt(HD)
    wq_p = (wq.reshape(H, HD, DIM)[:, perm, :] * scale).astype(np.float32)
    wk_p = wk.reshape(KVH, HD, DIM)[:, perm, :]
    cos_h = np.ascontiguousarray(freqs_cis[:, :, 0].T)  # [64, S]
    sin_h = np.ascontiguousarray(freqs_cis[:, :, 1].T)
    cosT = np.concatenate([cos_h, cos_h], axis=0).astype(bf)   # [128, S]
    sinT = np.concatenate([-sin_h, sin_h], axis=0).astype(bf)

    in_maps = []
    for c in range(N_CORES):
        b, g = c // KVH, c % KVH
        hq = slice(NH * g, NH * (g + 1))
        in_maps.append({
            "xT": np.ascontiguousarray(x[b].T).astype(bf),
            "wqT": np.ascontiguousarray(
                wq_p[hq].reshape(NH * HD, DIM).T).astype(bf),
            "wkT": np.ascontiguousarray(wk_p[g].T).astype(bf),
            "wvT": np.ascontiguousarray(wv[g * HD:(g + 1) * HD].T).astype(bf),
            "woT": np.ascontiguousarray(
                wo[:, NH * HD * g:NH * HD * (g + 1)].T).astype(bf),
            "cosT": cosT,
            "sinT": sinT,
        })
    return in_maps


def _reduce_outputs(results):
    out = np.zeros((B, S, DIM), np.float32)
    for c in range(N_CORES):
        out[c // KVH] += results[c]["out"].astype(np.float32)
    return out


def kernel(x, freqs_cis, wq, wk, wv, wo, _trace=False, _trace_kwargs=None):
    nc = _get_nc()
    in_maps = _prep_in_maps(x, freqs_cis, wq, wk, wv, wo)
    res = run_bass_kernel_spmd(nc, in_maps, core_ids=list(range(N_CORES)),
                               trace=_trace, **(_trace_kwargs or {}))
    out = _reduce_outputs(res.results)
    if _trace:
        _cached["last_exec_time_ns"] = res.exec_time_ns
        _cached["last_results"] = res
    return out


# revision 15
# speedup vs baseline: 1.0706x; 1.0277x over previous
"""GQA attention (B=2, S=2048, DIM=2048, H=16, KVH=4, HD=128, RoPE, causal)
on 8 TRN2 NeuronCores.

Sharding: core c -> batch b = c//4, head-group g = c%4 (q heads 4g..4g+3,
which map exactly to kv head g). Each core computes the partial output
attn_heads @ wo_slice.T  ([S, DIM]); the host sums the 4 partials per batch.

Device layout (everything "transposed", feature-major):
  xT   [DIM, S]   bf16   x[b].T
  wqT  [DIM, 512] bf16   (per-head even/odd-permuted, 1/sqrt(HD)-scaled) wq.T
  wkT  [DIM, 128] bf16   permuted wk.T
  wvT  [DIM, 128] bf16   wv.T (not permuted; v is not roped)
  woT  [512, DIM] bf16   wo[:, cols].T
  cosT/sinT [64, S] f32  rope tables, frequency-major

The per-head even/odd permutation (rows [0,2,..,126,1,3,..,127]) turns RoPE
pair-interleaving into contiguous half-partitions; q.k dot products are
invariant because q and k are permuted identically.

Attention is computed in transposed score layout: scoresT[k, q] so that
probsT feeds the PV matmul directly (lhsT = v natural layout), attnT falls
out in [hd, q] = exactly the lhsT the output projection needs, and the
softmax denominators come from an all-ones stationary matmul.
"""

import math
import sys
import types

import numpy as np

try:
    import concourse.bacc as bacc  # noqa: F401
except ImportError:
    sys.path.insert(0, "/opt/trn_rl_repo")

import ml_dtypes
import concourse.bacc as bacc
import concourse.tile as tile
from concourse import mybir
from concourse.bass_utils import run_bass_kernel_spmd
from concourse.bass import _add_dep_helper

BF16 = mybir.dt.bfloat16
F32 = mybir.dt.float32

B, S, DIM = 2, 2048, 2048
H, KVH, HD = 16, 4, 128
N_CORES = 8
P = 128
D_T = DIM // P      # 16 contraction tiles
NH = H // KVH       # 4 q-heads per core
QC = 512            # q-chunk (matmul moving free dim)
QB = S // QC        # 4 q-chunks
S_T = S // P        # 16 s-tiles / k-tiles
NEG = -1.0e9

_cached = {}


def _build_nc():
    nc = bacc.Bacc("TRN2", target_bir_lowering=False, debug=False,
                   num_devices=N_CORES)
    xT = nc.dram_tensor("xT", [DIM, S], BF16, kind="ExternalInput").ap()
    wqT = nc.dram_tensor("wqT", [DIM, NH * HD], BF16, kind="ExternalInput").ap()
    wkT = nc.dram_tensor("wkT", [DIM, HD], BF16, kind="ExternalInput").ap()
    wvT = nc.dram_tensor("wvT", [DIM, HD], BF16, kind="ExternalInput").ap()
    woT = nc.dram_tensor("woT", [NH * HD, DIM], BF16, kind="ExternalInput").ap()
    cosT = nc.dram_tensor("cosT", [HD, S], BF16, kind="ExternalInput").ap()
    sinT = nc.dram_tensor("sinT", [HD, S], BF16, kind="ExternalInput").ap()
    out = nc.dram_tensor("out", [S, DIM], BF16, kind="ExternalOutput").ap()

    with tile.TileContext(nc) as tc:
        _build_kernel(tc, xT, wqT, wkT, wvT, woT, cosT, sinT, out)
    nc.compile()
    return nc


def _build_kernel(tc, xT, wqT, wkT, wvT, woT, cosT, sinT, out):
    nc = tc.nc
    Exp = mybir.ActivationFunctionType.Exp

    with (
        tc.tile_pool(name="const", bufs=1) as const,
        tc.tile_pool(name="big", bufs=1) as big,
        tc.tile_pool(name="rtmp", bufs=8) as rtmp,
        tc.tile_pool(name="probs", bufs=8) as probs_pool,
        tc.tile_pool(name="attn", bufs=8) as attn_pool,
        tc.tile_pool(name="rz", bufs=2) as rz_pool,
        tc.tile_pool(name="osb", bufs=4) as osb_pool,
        tc.tile_pool(name="ps", bufs=4, space="PSUM") as ps_pool,
        tc.tile_pool(name="ps_at", bufs=2, space="PSUM") as ps_at_pool,
        tc.tile_pool(name="ps_z", bufs=2, space="PSUM") as ps_z_pool,
    ):
        # ---- constants ----
        ones = const.tile([P, P], BF16, name="ones")
        nc.vector.memset(ones, 1.0)
        # cos_sb = [cos; cos], sin_sb = [-sin; sin] (host-prepared), so the
        # whole rotation is 3 full-width ops on partition-aligned tiles.
        cos_sb = const.tile([HD, S], BF16, name="cos")
        sin_sb = const.tile([HD, S], BF16, name="sin")
        _dma_cs = []

        # ---- weights / activations ----
        # DMA priority: wk/wv (small, needed first) -> xT (gates everything)
        # -> wq (needed once Q-proj starts) -> cos/sin -> wo (needed ~100us
        # in). Deferred loads are dep-chained behind xT slices so they do
        # not steal HBM bandwidth from the critical path.
        wk_sb = big.tile([P, D_T, HD], BF16, name="wk")
        nc.sync.dma_start(out=wk_sb, in_=wkT.rearrange("(t p) j -> p t j", p=P))
        wv_sb = big.tile([P, D_T, HD], BF16, name="wv")
        nc.sync.dma_start(out=wv_sb, in_=wvT.rearrange("(t p) j -> p t j", p=P))

        xt_tiles = {}
        xt_dmas = []
        for dt in range(D_T):
            t = big.tile([P, S], BF16, name=f"xt{dt}")
            dma = nc.sync.dma_start(out=t, in_=xT[dt * P:(dt + 1) * P, :])
            if dt >= 8:
                _add_dep_helper(dma.ins, xt_dmas[dt - 8].ins, sync=True,
                                reason="stagger xT load")
            for sc in range(QB):
                xt_tiles[(dt, sc)] = t[:, sc * QC:(sc + 1) * QC]
            xt_dmas.append(dma)

        wq_sb = big.tile([P, D_T, NH * HD], BF16, name="wq")
        dma_wq = nc.sync.dma_start(out=wq_sb,
                                   in_=wqT.rearrange("(t p) j -> p t j", p=P))
        _add_dep_helper(dma_wq.ins, xt_dmas[4].ins, sync=True,
                        reason="wq after early xT")
        wo_sb = big.tile([P, NH, DIM], BF16, name="wo")
        dma_wo = nc.sync.dma_start(out=wo_sb,
                                   in_=woT.rearrange("(t p) d -> p t d", p=P))
        _add_dep_helper(dma_wo.ins, xt_dmas[15].ins, sync=True,
                        reason="wo after xT")

        for _src, _dst in ((cosT, cos_sb), (sinT, sin_sb)):
            _d = nc.sync.dma_start(out=_dst, in_=_src)
            _add_dep_helper(_d.ins, xt_dmas[8].ins, sync=True,
                            reason="rope tables after early xT")

        qT = big.tile([P, NH, S], BF16, name="qT")
        kT = big.tile([P, S], BF16, name="kT")
        v_sb = big.tile([P, S_T, HD], BF16, name="v")

        def rope(dst, ps, sc):
            """dst (bf16 [128,512] slice) <- rotate(ps).

            ACT stages ps to bf16 SBUF twice (straight + halves swapped via
            ScalarE partition-shifting copies); DVE then runs three
            full-width 16-bit 2x-mode ops against the sign-folded tables:
            dst = st*[cos;cos] + sw*[-sin;sin]."""
            h = HD // 2
            st = rtmp.tile([P, QC], BF16, name="rst")
            sw = rtmp.tile([P, QC], BF16, name="rsw")
            nc.scalar.copy(out=st, in_=ps)
            nc.scalar.copy(out=sw[0:h, :], in_=ps[h:P, :])
            nc.scalar.copy(out=sw[h:P, :], in_=ps[0:h, :])
            cos_c = cos_sb[:, sc * QC:(sc + 1) * QC]
            sin_c = sin_sb[:, sc * QC:(sc + 1) * QC]
            t0 = rtmp.tile([P, QC], BF16, name="rt")
            t1 = rtmp.tile([P, QC], BF16, name="rt")
            nc.vector.tensor_mul(t0, st, cos_c)
            nc.vector.tensor_mul(t1, sw, sin_c)
            nc.vector.tensor_add(dst, t0, t1)

        # K projection + rope
        for sc in range(QB):
            ps = ps_pool.tile([P, QC], F32, name="ps")
            for dt in range(D_T):
                nc.tensor.matmul(ps, lhsT=wk_sb[:, dt, :],
                                 rhs=xt_tiles[(dt, sc)],
                                 start=(dt == 0), stop=(dt == D_T - 1))
            rope(kT[:, sc * QC:(sc + 1) * QC], ps, sc)

        # Q projection + rope
        for hh in range(NH):
            for sc in range(QB):
                ps = ps_pool.tile([P, QC], F32, name="ps")
                for dt in range(D_T):
                    nc.tensor.matmul(ps, lhsT=wq_sb[:, dt, hh * HD:(hh + 1) * HD],
                                     rhs=xt_tiles[(dt, sc)],
                                     start=(dt == 0), stop=(dt == D_T - 1))
                rope(qT[:, hh, sc * QC:(sc + 1) * QC], ps, sc)

        # V projection (natural [s, hd] layout)
        for st in range(S_T):
            ps = ps_pool.tile([P, QC], F32, name="ps")
            for dt in range(D_T):
                nc.tensor.matmul(ps[:, 0:HD],
                                 lhsT=xt_tiles[(dt, st // 4)][:, (st % 4) * P:(st % 4 + 1) * P],
                                 rhs=wv_sb[:, dt, :],
                                 start=(dt == 0), stop=(dt == D_T - 1))
            nc.scalar.copy(out=v_sb[:, st, :], in_=ps[:, 0:HD])

        # ---- attention + output projection, per q-chunk ----
        for qb in reversed(range(QB)):
            nk = (qb + 1) * (QC // P)  # causal k-tiles for this q-chunk
            attn_tiles = []
            for hh in range(NH):
                at_ps = ps_at_pool.tile([P, QC], F32, name="at")
                z_ps = ps_z_pool.tile([P, QC], F32, name="z")
                for k in range(nk):
                    di = k - qb * (QC // P)
                    # On diagonal tiles only columns q0+128*di.. are causally
                    # valid; narrow every stage to that width. off=0 for all
                    # sub-diagonal tiles.
                    off = P * di if di > 0 else 0
                    w = QC - off
                    sc_ps = ps_pool.tile([P, QC], F32, name="ps")
                    nc.tensor.matmul(sc_ps[:, 0:w], lhsT=kT[:, k * P:(k + 1) * P],
                                     rhs=qT[:, hh, qb * QC + off:(qb + 1) * QC],
                                     start=True, stop=True)
                    pr = probs_pool.tile([P, QC], BF16, name="pr")
                    nc.scalar.activation(out=pr[:, 0:w], in_=sc_ps[:, 0:w],
                                         func=Exp)
                    if di >= 0:  # diagonal tile -> zero where c' < r
                        nc.gpsimd.affine_select(
                            out=pr[:, 0:w], in_=pr[:, 0:w],
                            compare_op=mybir.AluOpType.is_ge,
                            fill=0.0, base=0, pattern=[[1, w]],
                            channel_multiplier=-1)
                    nc.tensor.matmul(at_ps[:, off:QC], lhsT=v_sb[:, k, :],
                                     rhs=pr[:, 0:w],
                                     start=(k == 0), stop=(k == nk - 1))
                    nc.tensor.matmul(z_ps[:, off:QC], lhsT=ones, rhs=pr[:, 0:w],
                                     start=(k == 0), stop=(k == nk - 1))
                rz = rz_pool.tile([P, QC], F32, name="rz")
                nc.vector.reciprocal_approx_fast(out=rz, in_=z_ps)
                a_sb = attn_pool.tile([P, QC], BF16, name="attn")
                nc.vector.tensor_mul(a_sb, at_ps, rz)
                attn_tiles.append(a_sb)

            for st in range(QC // P):
                row0 = qb * QC + st * P
                for dc in range(DIM // QC):
                    op_ps = ps_pool.tile([P, QC], F32, name="ps")
                    for j in range(NH):
                        nc.tensor.matmul(
                            op_ps, lhsT=attn_tiles[j][:, st * P:(st + 1) * P],
                            rhs=wo_sb[:, j, dc * QC:(dc + 1) * QC],
                            start=(j == 0), stop=(j == NH - 1))
                    o_sb = osb_pool.tile([P, QC], BF16, name="osb")
                    if qb == 0 and (st * 4 + dc) >= 8:
                        nc.scalar.copy(out=o_sb, in_=op_ps)
                    else:
                        nc.vector.tensor_copy(out=o_sb, in_=op_ps)
                    nc.sync.dma_start(
                        out=out[row0:row0 + P, dc * QC:(dc + 1) * QC], in_=o_sb)


def _get_nc():
    if "nc" not in _cached:
        _cached["nc"] = _build_nc()
    return _cached["nc"]


def _prep_in_maps(x, freqs_cis, wq, wk, wv, wo):
    bf = ml_dtypes.bfloat16
    perm = np.concatenate([np.arange(0, HD, 2), np.arange(1, HD, 2)])
    scale = 1.0 / math.sqrt(HD)
    wq_p = (wq.reshape(H, HD, DIM)[:, perm, :] * scale).astype(np.float32)
    wk_p = wk.reshape(KVH, HD, DIM)[:, perm, :]
    cos_h = np.ascontiguousarray(freqs_cis[:, :, 0].T)  # [64, S]
    sin_h = np.ascontiguousarray(freqs_cis[:, :, 1].T)
    cosT = np.concatenate([cos_h, cos_h], axis=0).astype(bf)   # [128, S]
    sinT = np.concatenate([-sin_h, sin_h], axis=0).astype(bf)

    in_maps = []
    for c in range(N_CORES):
        b, g = c // KVH, c % KVH
        hq = slice(NH * g, NH * (g + 1))
        in_maps.append({
            "xT": np.ascontiguousarray(x[b].T).astype(bf),
            "wqT": np.ascontiguousarray(
                wq_p[hq].reshape(NH * HD, DIM).T).astype(bf),
            "wkT": np.ascontiguousarray(wk_p[g].T).astype(bf),
            "wvT": np.ascontiguousarray(wv[g * HD:(g + 1) * HD].T).astype(bf),
            "woT": np.ascontiguousarray(
                wo[:, NH * HD * g:NH * HD * (g + 1)].T).astype(bf),
            "cosT": cosT,
            "sinT": sinT,
        })
    return in_maps


def _reduce_outputs(results):
    out = np.zeros((B, S, DIM), np.float32)
    for c in range(N_CORES):
        out[c // KVH] += results[c]["out"].astype(np.float32)
    return out


def kernel(x, freqs_cis, wq, wk, wv, wo, _trace=False, _trace_kwargs=None):
    nc = _get_nc()
    in_maps = _prep_in_maps(x, freqs_cis, wq, wk, wv, wo)
    res = run_bass_kernel_spmd(nc, in_maps, core_ids=list(range(N_CORES)),
                               trace=_trace, **(_trace_kwargs or {}))
    out = _reduce_outputs(res.results)
    if _trace:
        _cached["last_exec_time_ns"] = res.exec_time_ns
        _cached["last_results"] = res
    return out


# revision 16
# speedup vs baseline: 1.0831x; 1.0117x over previous
"""GQA attention (B=2, S=2048, DIM=2048, H=16, KVH=4, HD=128, RoPE, causal)
on 8 TRN2 NeuronCores.

Sharding: core c -> batch b = c//4, head-group g = c%4 (q heads 4g..4g+3,
which map exactly to kv head g). Each core computes the partial output
attn_heads @ wo_slice.T  ([S, DIM]); the host sums the 4 partials per batch.

Device layout (everything "transposed", feature-major):
  xT   [DIM, S]   bf16   x[b].T
  wqT  [DIM, 512] bf16   (per-head even/odd-permuted, 1/sqrt(HD)-scaled) wq.T
  wkT  [DIM, 128] bf16   permuted wk.T
  wvT  [DIM, 128] bf16   wv.T (not permuted; v is not roped)
  woT  [512, DIM] bf16   wo[:, cols].T
  cosT/sinT [64, S] f32  rope tables, frequency-major

The per-head even/odd permutation (rows [0,2,..,126,1,3,..,127]) turns RoPE
pair-interleaving into contiguous half-partitions; q.k dot products are
invariant because q and k are permuted identically.

Attention is computed in transposed score layout: scoresT[k, q] so that
probsT feeds the PV matmul directly (lhsT = v natural layout), attnT falls
out in [hd, q] = exactly the lhsT the output projection needs, and the
softmax denominators come from an all-ones stationary matmul.
"""

import math
import sys
import types

import numpy as np

try:
    import concourse.bacc as bacc  # noqa: F401
except ImportError:
    sys.path.insert(0, "/opt/trn_rl_repo")

import ml_dtypes
import concourse.bacc as bacc
import concourse.tile as tile
from concourse import mybir
from concourse.bass_utils import run_bass_kernel_spmd
from concourse.bass import _add_dep_helper

BF16 = mybir.dt.bfloat16
F32 = mybir.dt.float32

B, S, DIM = 2, 2048, 2048
H, KVH, HD = 16, 4, 128
N_CORES = 8
P = 128
D_T = DIM // P      # 16 contraction tiles
NH = H // KVH       # 4 q-heads per core
QC = 512            # q-chunk (matmul moving free dim)
QB = S // QC        # 4 q-chunks
S_T = S // P        # 16 s-tiles / k-tiles
NEG = -1.0e9

_cached = {}


def _build_nc():
    nc = bacc.Bacc("TRN2", target_bir_lowering=False, debug=False,
                   num_devices=N_CORES)
    xT = nc.dram_tensor("xT", [DIM, S], BF16, kind="ExternalInput").ap()
    wqT = nc.dram_tensor("wqT", [DIM, NH * HD], BF16, kind="ExternalInput").ap()
    wkT = nc.dram_tensor("wkT", [DIM, HD], BF16, kind="ExternalInput").ap()
    wvT = nc.dram_tensor("wvT", [DIM, HD], BF16, kind="ExternalInput").ap()
    woT = nc.dram_tensor("woT", [NH * HD, DIM], BF16, kind="ExternalInput").ap()
    cosT = nc.dram_tensor("cosT", [HD, S], BF16, kind="ExternalInput").ap()
    sinT = nc.dram_tensor("sinT", [HD, S], BF16, kind="ExternalInput").ap()
    out = nc.dram_tensor("out", [S, DIM], BF16, kind="ExternalOutput").ap()

    with tile.TileContext(nc) as tc:
        _build_kernel(tc, xT, wqT, wkT, wvT, woT, cosT, sinT, out)
    nc.compile()
    return nc


def _build_kernel(tc, xT, wqT, wkT, wvT, woT, cosT, sinT, out):
    nc = tc.nc
    Exp = mybir.ActivationFunctionType.Exp

    with (
        tc.tile_pool(name="const", bufs=1) as const,
        tc.tile_pool(name="big", bufs=1) as big,
        tc.tile_pool(name="rtmp", bufs=12) as rtmp,
        tc.tile_pool(name="probs", bufs=8) as probs_pool,
        tc.tile_pool(name="attn", bufs=8) as attn_pool,
        tc.tile_pool(name="rz", bufs=2) as rz_pool,
        tc.tile_pool(name="osb", bufs=4) as osb_pool,
        tc.tile_pool(name="ps", bufs=4, space="PSUM") as ps_pool,
        tc.tile_pool(name="ps_at", bufs=2, space="PSUM") as ps_at_pool,
        tc.tile_pool(name="ps_z", bufs=2, space="PSUM") as ps_z_pool,
    ):
        # ---- constants ----
        ones = const.tile([P, P], BF16, name="ones")
        nc.vector.memset(ones, 1.0)
        # cos_sb = [cos; cos], sin_sb = [-sin; sin] (host-prepared), so the
        # whole rotation is 3 full-width ops on partition-aligned tiles.
        cos_sb = const.tile([HD, S], BF16, name="cos")
        sin_sb = const.tile([HD, S], BF16, name="sin")
        _dma_cs = []

        # ---- weights / activations ----
        # DMA priority: wk/wv (small, needed first) -> xT (gates everything)
        # -> wq (needed once Q-proj starts) -> cos/sin -> wo (needed ~100us
        # in). Deferred loads are dep-chained behind xT slices so they do
        # not steal HBM bandwidth from the critical path.
        wk_sb = big.tile([P, D_T, HD], BF16, name="wk")
        nc.sync.dma_start(out=wk_sb, in_=wkT.rearrange("(t p) j -> p t j", p=P))
        wv_sb = big.tile([P, D_T, HD], BF16, name="wv")
        nc.sync.dma_start(out=wv_sb, in_=wvT.rearrange("(t p) j -> p t j", p=P))

        xt_tiles = {}
        xt_dmas = []
        for dt in range(D_T):
            t = big.tile([P, S], BF16, name=f"xt{dt}")
            dma = nc.sync.dma_start(out=t, in_=xT[dt * P:(dt + 1) * P, :])
            if dt >= 8:
                _add_dep_helper(dma.ins, xt_dmas[dt - 8].ins, sync=True,
                                reason="stagger xT load")
            for sc in range(QB):
                xt_tiles[(dt, sc)] = t[:, sc * QC:(sc + 1) * QC]
            xt_dmas.append(dma)

        wq_sb = big.tile([P, D_T, NH * HD], BF16, name="wq")
        wq_r = wqT.rearrange("(t p) j -> p t j", p=P)
        for hh in range(NH):
            dma_wq = nc.sync.dma_start(
                out=wq_sb[:, :, hh * HD:(hh + 1) * HD],
                in_=wq_r[:, :, hh * HD:(hh + 1) * HD])
            _add_dep_helper(dma_wq.ins, xt_dmas[2 + 2 * hh].ins, sync=True,
                            reason="wq chunk after early xT")
        wo_sb = big.tile([P, NH, DIM], BF16, name="wo")
        dma_wo = nc.sync.dma_start(out=wo_sb,
                                   in_=woT.rearrange("(t p) d -> p t d", p=P))
        _add_dep_helper(dma_wo.ins, xt_dmas[15].ins, sync=True,
                        reason="wo after xT")

        for _src, _dst in ((cosT, cos_sb), (sinT, sin_sb)):
            _d = nc.sync.dma_start(out=_dst, in_=_src)
            _add_dep_helper(_d.ins, xt_dmas[8].ins, sync=True,
                            reason="rope tables after early xT")

        qT = big.tile([P, NH, S], BF16, name="qT")
        kT = big.tile([P, S], BF16, name="kT")
        v_sb = big.tile([P, S_T, HD], BF16, name="v")

        def rope(dst, ps, sc):
            """dst (bf16 [128,512] slice) <- rotate(ps).

            ACT stages ps to bf16 SBUF twice (straight + halves swapped via
            ScalarE partition-shifting copies); DVE then runs three
            full-width 16-bit 2x-mode ops against the sign-folded tables:
            dst = st*[cos;cos] + sw*[-sin;sin]."""
            h = HD // 2
            st = rtmp.tile([P, QC], BF16, name="rst")
            sw = rtmp.tile([P, QC], BF16, name="rsw")
            nc.scalar.copy(out=st, in_=ps)
            nc.scalar.copy(out=sw[0:h, :], in_=ps[h:P, :])
            nc.scalar.copy(out=sw[h:P, :], in_=ps[0:h, :])
            cos_c = cos_sb[:, sc * QC:(sc + 1) * QC]
            sin_c = sin_sb[:, sc * QC:(sc + 1) * QC]
            t0 = rtmp.tile([P, QC], BF16, name="rt")
            t1 = rtmp.tile([P, QC], BF16, name="rt")
            nc.vector.tensor_mul(t0, st, cos_c)
            nc.vector.tensor_mul(t1, sw, sin_c)
            nc.vector.tensor_add(dst, t0, t1)

        # K projection + rope
        for sc in range(QB):
            ps = ps_pool.tile([P, QC], F32, name="ps")
            for dt in range(D_T):
                nc.tensor.matmul(ps, lhsT=wk_sb[:, dt, :],
                                 rhs=xt_tiles[(dt, sc)],
                                 start=(dt == 0), stop=(dt == D_T - 1))
            rope(kT[:, sc * QC:(sc + 1) * QC], ps, sc)

        # Q projection + rope
        for hh in range(NH):
            for sc in range(QB):
                ps = ps_pool.tile([P, QC], F32, name="ps")
                for dt in range(D_T):
                    nc.tensor.matmul(ps, lhsT=wq_sb[:, dt, hh * HD:(hh + 1) * HD],
                                     rhs=xt_tiles[(dt, sc)],
                                     start=(dt == 0), stop=(dt == D_T - 1))
                rope(qT[:, hh, sc * QC:(sc + 1) * QC], ps, sc)

        # V projection (natural [s, hd] layout)
        for st in range(S_T):
            ps = ps_pool.tile([P, QC], F32, name="ps")
            for dt in range(D_T):
                nc.tensor.matmul(ps[:, 0:HD],
                                 lhsT=xt_tiles[(dt, st // 4)][:, (st % 4) * P:(st % 4 + 1) * P],
                                 rhs=wv_sb[:, dt, :],
                                 start=(dt == 0), stop=(dt == D_T - 1))
            nc.scalar.copy(out=v_sb[:, st, :], in_=ps[:, 0:HD])

        # ---- attention + output projection, per q-chunk ----
        for qb in reversed(range(QB)):
            nk = (qb + 1) * (QC // P)  # causal k-tiles for this q-chunk
            attn_tiles = []
            for hh in range(NH):
                at_ps = ps_at_pool.tile([P, QC], F32, name="at")
                z_ps = ps_z_pool.tile([P, QC], F32, name="z")
                for k in range(nk):
                    di = k - qb * (QC // P)
                    # On diagonal tiles only columns q0+128*di.. are causally
                    # valid; narrow every stage to that width. off=0 for all
                    # sub-diagonal tiles.
                    off = P * di if di > 0 else 0
                    w = QC - off
                    sc_ps = ps_pool.tile([P, QC], F32, name="ps")
                    nc.tensor.matmul(sc_ps[:, 0:w], lhsT=kT[:, k * P:(k + 1) * P],
                                     rhs=qT[:, hh, qb * QC + off:(qb + 1) * QC],
                                     start=True, stop=True)
                    pr = probs_pool.tile([P, QC], BF16, name="pr")
                    nc.scalar.activation(out=pr[:, 0:w], in_=sc_ps[:, 0:w],
                                         func=Exp)
                    if di >= 0:  # diagonal tile -> zero where c' < r
                        nc.gpsimd.affine_select(
                            out=pr[:, 0:w], in_=pr[:, 0:w],
                            compare_op=mybir.AluOpType.is_ge,
                            fill=0.0, base=0, pattern=[[1, w]],
                            channel_multiplier=-1)
                    nc.tensor.matmul(at_ps[:, off:QC], lhsT=v_sb[:, k, :],
                                     rhs=pr[:, 0:w],
                                     start=(k == 0), stop=(k == nk - 1))
                    nc.tensor.matmul(z_ps[:, off:QC], lhsT=ones, rhs=pr[:, 0:w],
                                     start=(k == 0), stop=(k == nk - 1))
                rz = rz_pool.tile([P, QC], F32, name="rz")
                nc.vector.reciprocal_approx_fast(out=rz, in_=z_ps)
                a_sb = attn_pool.tile([P, QC], BF16, name="attn")
                nc.vector.tensor_mul(a_sb, at_ps, rz)
                attn_tiles.append(a_sb)

            for st in range(QC // P):
                row0 = qb * QC + st * P
                for dc in range(DIM // QC):
                    op_ps = ps_pool.tile([P, QC], F32, name="ps")
                    for j in range(NH):
                        nc.tensor.matmul(
                            op_ps, lhsT=attn_tiles[j][:, st * P:(st + 1) * P],
                            rhs=wo_sb[:, j, dc * QC:(dc + 1) * QC],
                            start=(j == 0), stop=(j == NH - 1))
                    o_sb = osb_pool.tile([P, QC], BF16, name="osb")
                    if qb == 0 and (st * 4 + dc) % 2 == 0:
                        nc.scalar.copy(out=o_sb, in_=op_ps)
                    else:
                        nc.vector.tensor_copy(out=o_sb, in_=op_ps)
                    nc.sync.dma_start(
                        out=out[row0:row0 + P, dc * QC:(dc + 1) * QC], in_=o_sb)


def _get_nc():
    if "nc" not in _cached:
        _cached["nc"] = _build_nc()
    return _cached["nc"]


def _prep_in_maps(x, freqs_cis, wq, wk, wv, wo):
    bf = ml_dtypes.bfloat16
    perm = np.concatenate([np.arange(0, HD, 2), np.arange(1, HD, 2)])
    scale = 1.0 / math.sqrt(HD)
    wq_p = (wq.reshape(H, HD, DIM)[:, perm, :] * scale).astype(np.float32)
    wk_p = wk.reshape(KVH, HD, DIM)[:, perm, :]
    cos_h = np.ascontiguousarray(freqs_cis[:, :, 0].T)  # [64, S]
    sin_h = np.ascontiguousarray(freqs_cis[:, :, 1].T)
    cosT = np.concatenate([cos_h, cos_h], axis=0).astype(bf)   # [128, S]
    sinT = np.concatenate([-sin_h, sin_h], axis=0).astype(bf)

    in_maps = []
    for c in range(N_CORES):
        b, g = c // KVH, c % KVH
        hq = slice(NH * g, NH * (g + 1))
        in_maps.append({
            "xT": np.ascontiguousarray(x[b].T).astype(bf),
            "wqT": np.ascontiguousarray(
                wq_p[hq].reshape(NH * HD, DIM).T).astype(bf),
            "wkT": np.ascontiguousarray(wk_p[g].T).astype(bf),
            "wvT": np.ascontiguousarray(wv[g * HD:(g + 1) * HD].T).astype(bf),
            "woT": np.ascontiguousarray(
                wo[:, NH * HD * g:NH * HD * (g + 1)].T).astype(bf),
            "cosT": cosT,
            "sinT": sinT,
        })
    return in_maps


def _reduce_outputs(results):
    out = np.zeros((B, S, DIM), np.float32)
    for c in range(N_CORES):
        out[c // KVH] += results[c]["out"].astype(np.float32)
    return out


def kernel(x, freqs_cis, wq, wk, wv, wo, _trace=False, _trace_kwargs=None):
    nc = _get_nc()
    in_maps = _prep_in_maps(x, freqs_cis, wq, wk, wv, wo)
    res = run_bass_kernel_spmd(nc, in_maps, core_ids=list(range(N_CORES)),
                               trace=_trace, **(_trace_kwargs or {}))
    out = _reduce_outputs(res.results)
    if _trace:
        _cached["last_exec_time_ns"] = res.exec_time_ns
        _cached["last_results"] = res
    return out
